# revision 1
# baseline (speedup 1.0000x reference)
"""Mamba SSM block on 8 TRN2 NeuronCores (Bass/Tile, SPMD).

Sharding: d_inner (2048 -> 256/core) across in_proj, conv, dt/B/C projections
and the selective scan (all per-core local). Two small collectives per
512-token chunk:
  - AllReduce of x_dbl projection partials [96, 512] fp32
  - AllGather of the gated scan output yg [256, 512] bf16 (pure copy), after
    which each core computes only its 128 rows of the (host-folded)
    W_c = W_out @ W_out_ssm output projection.

Scan: h[t] = exp(A dt[t]) h[t-1] + (dt[t] x[t]) B[t] via 16 independent
tensor_tensor_scan ops (one per state index) along tokens, chained across
chunks through per-partition `initial` APs. Matmul operands and scan tensors
are bf16 (rel_err ~4e-3 validated vs fp32 reference); conv, activations, dt
and the AllReduce stay fp32.
"""
import numpy as np
import ml_dtypes

import concourse.bass as bass
import concourse.tile as tile
from concourse import bacc, mybir
from concourse.bass_utils import run_bass_kernel_spmd

BFnp = ml_dtypes.bfloat16
F32 = mybir.dt.float32
BF16 = mybir.dt.bfloat16
AF = mybir.ActivationFunctionType
OP = mybir.AluOpType

NC = 8
B, L, DM = 2, 2048, 1024
DI, S, R, KC = 2048, 16, 64, 4
DIL = DI // NC            # 256 d_inner per core
NT = B * L                # 4096 tokens (batch-major)
TC = 512                  # tokens per chunk
NCH = NT // TC            # 8 chunks
EL = DM // NC             # 128 output rows per core
NI = DIL // 128           # 2 partition tiles of local d_inner

_NC_CACHE = {}


def build():
    if "nc" in _NC_CACHE:
        return _NC_CACHE["nc"]
    nc = bacc.Bacc("TRN2", target_bir_lowering=False, debug=False, num_devices=NC)

    # ---- per-core DRAM inputs (host pre-sharded / transposed / casted) ----
    x_t = nc.dram_tensor("x_t", [DM, NT], BF16, kind="ExternalInput")         # replicated
    w_in_x = nc.dram_tensor("w_in_x", [DM, DIL], BF16, kind="ExternalInput")  # W_in[dk,:].T
    w_in_z = nc.dram_tensor("w_in_z", [DM, DIL], BF16, kind="ExternalInput")
    conv_w = nc.dram_tensor("conv_w", [DIL, KC], F32, kind="ExternalInput")
    conv_b = nc.dram_tensor("conv_b", [DIL, 1], F32, kind="ExternalInput")
    w_xp = nc.dram_tensor("w_xp", [DIL, R + 2 * S], BF16, kind="ExternalInput")  # W_xp[:,dk].T
    w_dt = nc.dram_tensor("w_dt", [R, DIL], BF16, kind="ExternalInput")          # W_dt[dk,:].T
    b_dt = nc.dram_tensor("b_dt", [DIL, 1], F32, kind="ExternalInput")
    a_mat = nc.dram_tensor("a_mat", [DIL, S], F32, kind="ExternalInput")        # -exp(A_log[dk])
    d_vec = nc.dram_tensor("d_vec", [DIL, 1], F32, kind="ExternalInput")
    w_c = nc.dram_tensor("w_c", [DI, EL], BF16, kind="ExternalInput")           # W_c[ek,:].T
    b_o = nc.dram_tensor("b_o", [EL, 1], F32, kind="ExternalInput")
    sel_in = nc.dram_tensor("sel_in", [S, S * 128], BF16, kind="ExternalInput")
    out = nc.dram_tensor("out", [NCH, EL, TC], F32, kind="ExternalOutput")

    with tile.TileContext(nc) as tc:
        with (
            tc.tile_pool(name="wpool", bufs=1) as wp,     # persistent weights
            tc.tile_pool(name="xpool", bufs=2) as xp,
            tc.tile_pool(name="xckp", bufs=1) as xkp,     # streamed x / yg gather
            tc.tile_pool(name="work", bufs=1) as wk,      # DVE-only transients
            tc.tile_pool(name="worka", bufs=2) as wka,    # ACT/DMA-written tiles
            tc.tile_pool(name="keep", bufs=6) as kp,      # xs/g (live across chunk)
            tc.tile_pool(name="scan", bufs=1) as sc,      # big bf16 scan tiles
            tc.tile_pool(name="scana", bufs=2) as sca,    # a (ACT-written, dbl buf)
            tc.tile_pool(name="bcast", bufs=1) as bcp,    # B/C broadcast tiles
            tc.tile_pool(name="state", bufs=1) as st,     # persistent hprev/xtail
            tc.tile_pool(name="psA", bufs=2, space="PSUM") as psA,
            tc.tile_pool(name="psB", bufs=1, space="PSUM") as psB,
            tc.tile_pool(name="dram", bufs=4, space="DRAM") as dr,
        ):
            # ---------- load weights ----------
            winx = wp.tile([128, 8 * NI * 128], BF16, tag="winx")
            winz = wp.tile([128, 8 * NI * 128], BF16, tag="winz")
            for kt in range(8):
                for i in range(NI):
                    nc.sync.dma_start(
                        winx[:, (kt * NI + i) * 128:(kt * NI + i + 1) * 128],
                        w_in_x[kt * 128:(kt + 1) * 128, i * 128:(i + 1) * 128])
                    nc.sync.dma_start(
                        winz[:, (kt * NI + i) * 128:(kt * NI + i + 1) * 128],
                        w_in_z[kt * 128:(kt + 1) * 128, i * 128:(i + 1) * 128])
            wxp = wp.tile([128, NI * (R + 2 * S)], BF16, tag="wxp")
            for i in range(NI):
                nc.sync.dma_start(
                    wxp[:, i * (R + 2 * S):(i + 1) * (R + 2 * S)],
                    w_xp[i * 128:(i + 1) * 128, :])
            wdt = wp.tile([R, NI * 128], BF16, tag="wdt")
            nc.sync.dma_start(wdt[:], w_dt[:, :])
            wc = wp.tile([128, 16 * EL], BF16, tag="wc")
            for kt in range(16):
                nc.sync.dma_start(
                    wc[:, kt * EL:(kt + 1) * EL],
                    w_c[kt * 128:(kt + 1) * 128, :])
            cw = wp.tile([128, NI * KC], F32, tag="cw")
            cb = wp.tile([128, NI], F32, tag="cb")
            bdt = wp.tile([128, NI], F32, tag="bdt")
            dv = wp.tile([128, NI], F32, tag="dv")
            am = wp.tile([128, NI * S], F32, tag="am")
            for i in range(NI):
                sl = slice(i * 128, (i + 1) * 128)
                nc.sync.dma_start(cw[:, i * KC:(i + 1) * KC], conv_w[sl, :])
                nc.sync.dma_start(cb[:, i:i + 1], conv_b[sl, :])
                nc.sync.dma_start(bdt[:, i:i + 1], b_dt[sl, :])
                nc.sync.dma_start(dv[:, i:i + 1], d_vec[sl, :])
                nc.sync.dma_start(am[:, i * S:(i + 1) * S], a_mat[sl, :])
            bo = wp.tile([EL, 1], F32, tag="bo")
            nc.sync.dma_start(bo[:], b_o[:, :])
            sel = wp.tile([S, S * 128], BF16, tag="sel")
            nc.sync.dma_start(sel[:], sel_in[:, :])

            hprev = st.tile([128, NI * S], BF16, tag="hprev")
            xtail = st.tile([128, NI * 3], F32, tag="xtail")

            ctx = {}

            def front(c):
                """x stream, in_proj, conv, silu, z-gate, x_dbl partial, AR issue."""
                t0 = c * TC
                reset = (c % (NCH // B) == 0)

                xck = xkp.tile([128, 8 * TC], BF16, tag="xck")
                for kt in range(8):
                    nc.sync.dma_start(
                        xck[:, kt * TC:(kt + 1) * TC],
                        x_t[kt * 128:(kt + 1) * 128, t0:t0 + TC])

                xs_i, g_i = [], []
                for i in range(NI):
                    ps_x = psA.tile([128, TC], F32, tag="psx")
                    ps_z = psA.tile([128, TC], F32, tag="psz")
                    for kt in range(8):
                        wsl = slice((kt * NI + i) * 128, (kt * NI + i + 1) * 128)
                        nc.tensor.matmul(ps_x[:], winx[:, wsl], xck[:, kt * TC:(kt + 1) * TC],
                                         start=(kt == 0), stop=(kt == 7))
                    for kt in range(8):
                        wsl = slice((kt * NI + i) * 128, (kt * NI + i + 1) * 128)
                        nc.tensor.matmul(ps_z[:], winz[:, wsl], xck[:, kt * TC:(kt + 1) * TC],
                                         start=(kt == 0), stop=(kt == 7))

                    # causal depthwise conv (taps read PSUM)
                    head = wk.tile([128, 7], F32, tag="head")
                    if reset:
                        nc.gpsimd.memset(head[:, 0:3], 0.0)
                    else:
                        nc.vector.tensor_copy(head[:, 0:3], xtail[:, i * 3:i * 3 + 3])
                    nc.vector.tensor_copy(head[:, 3:7], ps_x[:, 0:4])
                    nc.vector.tensor_copy(xtail[:, i * 3:i * 3 + 3], ps_x[:, TC - 3:TC])

                    acc0 = wk.tile([128, TC], F32, tag="accA")
                    nc.vector.tensor_scalar_mul(acc0[:, 3:], ps_x[:, 0:TC - 3], cw[:, i * KC:i * KC + 1])
                    acc1 = wk.tile([128, TC], F32, tag="accB")
                    nc.vector.scalar_tensor_tensor(
                        out=acc1[:, 3:], in0=ps_x[:, 1:TC - 2], scalar=cw[:, i * KC + 1:i * KC + 2],
                        in1=acc0[:, 3:], op0=OP.mult, op1=OP.add)
                    acc2 = wk.tile([128, TC], F32, tag="accA")
                    nc.vector.scalar_tensor_tensor(
                        out=acc2[:, 3:], in0=ps_x[:, 2:TC - 1], scalar=cw[:, i * KC + 2:i * KC + 3],
                        in1=acc1[:, 3:], op0=OP.mult, op1=OP.add)
                    u = wk.tile([128, TC], F32, tag="accB")
                    nc.vector.scalar_tensor_tensor(
                        out=u[:, 3:], in0=ps_x[:, 3:TC], scalar=cw[:, i * KC + 3:i * KC + 4],
                        in1=acc2[:, 3:], op0=OP.mult, op1=OP.add)
                    nc.vector.tensor_scalar_mul(acc0[:, 0:3], head[:, 0:3], cw[:, i * KC:i * KC + 1])
                    nc.vector.scalar_tensor_tensor(
                        out=acc1[:, 0:3], in0=head[:, 1:4], scalar=cw[:, i * KC + 1:i * KC + 2],
                        in1=acc0[:, 0:3], op0=OP.mult, op1=OP.add)
                    nc.vector.scalar_tensor_tensor(
                        out=acc2[:, 0:3], in0=head[:, 2:5], scalar=cw[:, i * KC + 2:i * KC + 3],
                        in1=acc1[:, 0:3], op0=OP.mult, op1=OP.add)
                    nc.vector.scalar_tensor_tensor(
                        out=u[:, 0:3], in0=head[:, 3:6], scalar=cw[:, i * KC + 3:i * KC + 4],
                        in1=acc2[:, 0:3], op0=OP.mult, op1=OP.add)
                    sgu = wka.tile([128, TC], F32, tag="act1")
                    nc.scalar.activation(sgu[:], u[:], AF.Sigmoid, bias=cb[:, i:i + 1])
                    xs = kp.tile([128, TC], BF16, tag="xs")
                    nc.vector.scalar_tensor_tensor(
                        out=xs[:], in0=u[:], scalar=cb[:, i:i + 1], in1=sgu[:],
                        op0=OP.add, op1=OP.mult)
                    xs_i.append(xs)

                    # z gate: g = z * sigmoid(z)
                    sgz = wka.tile([128, TC], F32, tag="act3")
                    nc.scalar.activation(sgz[:], ps_z[:], AF.Sigmoid)
                    g = kp.tile([128, TC], BF16, tag="g")
                    nc.vector.scalar_tensor_tensor(
                        out=g[:], in0=sgz[:], scalar=1.0, in1=ps_z[:],
                        op0=OP.mult, op1=OP.mult)
                    g_i.append(g)

                # x_dbl partial + AllReduce
                ps_xd = psB.tile([R + 2 * S, TC], F32, tag="psxd")
                for i in range(NI):
                    nc.tensor.matmul(ps_xd[:], wxp[:, i * (R + 2 * S):(i + 1) * (R + 2 * S)],
                                     xs_i[i][:], start=(i == 0), stop=(i == NI - 1))
                xd_sb = wka.tile([R + 2 * S, TC], BF16, tag="xdsb")
                nc.scalar.copy(xd_sb[:], ps_xd[:])
                xd_part = dr.tile([R + 2 * S, TC], BF16, tag="xdp")
                nc.sync.dma_start(xd_part[:], xd_sb[:])
                xd_red = nc.dram_tensor(f"xd_red_{c}", [R + 2 * S, TC], BF16, addr_space="Shared")
                nc.gpsimd.collective_compute(
                    "AllReduce", OP.add, replica_groups=[list(range(NC))],
                    ins=[xd_part[:]], outs=[xd_red.ap()])
                ctx[c] = dict(xs_i=xs_i, g_i=g_i, xd_red=xd_red)

            def mid(c):
                """dtr/B/C loads, dt, a-exps, bb, scan, hc, tree, gate, yg, AG issue."""
                reset = (c % (NCH // B) == 0)
                xs_i = ctx[c]["xs_i"]
                g_i = ctx[c]["g_i"]
                xd_red = ctx[c]["xd_red"]

                dtr = wka.tile([R, TC], BF16, tag="dtr")
                nc.sync.dma_start(dtr[:], xd_red.ap()[0:R, :])
                brows = wka.tile([S, TC], BF16, tag="brows")
                nc.sync.dma_start(brows[:], xd_red.ap()[R:R + S, :])
                crows = wka.tile([S, TC], BF16, tag="crows")
                nc.sync.dma_start(crows[:], xd_red.ap()[R + S:R + 2 * S, :])
                b_bc = bcp.tile([128, S * TC], BF16, tag="bbc")
                c_bc = bcp.tile([128, S * TC], BF16, tag="cbc")
                for s in range(S):
                    ps_bc = psB.tile([128, TC], F32, tag="psbc")
                    nc.tensor.matmul(ps_bc[:], sel[:, s * 128:(s + 1) * 128],
                                     brows[:], start=True, stop=True)
                    nc.scalar.copy(b_bc[:, s * TC:(s + 1) * TC], ps_bc[:])
                    ps_cc = psB.tile([128, TC], F32, tag="psbc")
                    nc.tensor.matmul(ps_cc[:], sel[:, s * 128:(s + 1) * 128],
                                     crows[:], start=True, stop=True)
                    nc.scalar.copy(c_bc[:, s * TC:(s + 1) * TC], ps_cc[:])

                yg_part = dr.tile([DIL, TC], BF16, tag="ygp")
                for i in range(NI):
                    ps_dt = psB.tile([128, TC], F32, tag="psdt")
                    nc.tensor.matmul(ps_dt[:], wdt[:, i * 128:(i + 1) * 128], dtr[:],
                                     start=True, stop=True)
                    edt = wka.tile([128, TC], F32, tag="act2")
                    nc.scalar.activation(edt[:], ps_dt[:], AF.Exp, bias=bdt[:, i:i + 1])
                    dt = wka.tile([128, TC], F32, tag="dtt")
                    nc.scalar.activation(dt[:], edt[:], AF.Ln, bias=1.0)

                    dtx = wk.tile([128, TC], BF16, tag="dtx")
                    nc.vector.tensor_tensor(out=dtx[:], in0=dt[:], in1=xs_i[i][:], op=OP.mult)

                    a_t = sca.tile([128, S * TC], BF16, tag="a_t")
                    for s in range(S):
                        nc.scalar.activation(
                            a_t[:, s * TC:(s + 1) * TC], dt[:], AF.Exp,
                            scale=am[:, i * S + s:i * S + s + 1])

                    bb_t = sc.tile([128, S * TC], BF16, tag="bb_t")
                    nc.vector.tensor_tensor(
                        out=bb_t[:].rearrange("p (s t) -> p s t", s=S),
                        in0=dtx[:].unsqueeze(1).broadcast_to([128, S, TC]),
                        in1=b_bc[:].rearrange("p (s t) -> p s t", s=S), op=OP.mult)

                    h_t = sc.tile([128, S * TC], BF16, tag="h_t")
                    if reset:
                        nc.gpsimd.memset(hprev[:, i * S:(i + 1) * S], 0.0)
                    for s in range(S):
                        nc.vector.tensor_tensor_scan(
                            h_t[:, s * TC:(s + 1) * TC],
                            a_t[:, s * TC:(s + 1) * TC],
                            bb_t[:, s * TC:(s + 1) * TC],
                            hprev[:, i * S + s:i * S + s + 1],
                            op0=OP.mult, op1=OP.add)
                    nc.sync.dma_start(
                        hprev[:, i * S:(i + 1) * S],
                        h_t[:].rearrange("p (s t) -> p s t", s=S)[:, :, TC - 1])

                    hc_t = sc.tile([128, S * TC], BF16, tag="hc_t")
                    nc.vector.tensor_tensor(out=hc_t[:], in0=h_t[:], in1=c_bc[:], op=OP.mult)
                    r1 = sc.tile([128, S * TC // 2], BF16, tag="bb_t")
                    nc.gpsimd.tensor_tensor(out=r1[:], in0=hc_t[:, :S * TC // 2],
                                            in1=hc_t[:, S * TC // 2:], op=OP.add)
                    r2 = sc.tile([128, S * TC // 4], BF16, tag="h_t")
                    nc.gpsimd.tensor_tensor(out=r2[:], in0=r1[:, :S * TC // 4],
                                            in1=r1[:, S * TC // 4:], op=OP.add)
                    r3 = sc.tile([128, S * TC // 8], BF16, tag="bb_t")
                    nc.vector.tensor_tensor(out=r3[:], in0=r2[:, :S * TC // 8],
                                            in1=r2[:, S * TC // 8:], op=OP.add)
                    y = wk.tile([128, TC], F32, tag="y")
                    nc.vector.tensor_tensor(out=y[:], in0=r3[:, :TC], in1=r3[:, TC:], op=OP.add)

                    yD = wk.tile([128, TC], F32, tag="yD")
                    nc.vector.scalar_tensor_tensor(
                        out=yD[:], in0=xs_i[i][:], scalar=dv[:, i:i + 1], in1=y[:],
                        op0=OP.mult, op1=OP.add)
                    yg = wk.tile([128, TC], BF16, tag="yg")
                    nc.vector.tensor_tensor(out=yg[:], in0=yD[:], in1=g_i[i][:], op=OP.mult)
                    nc.sync.dma_start(yg_part[i * 128:(i + 1) * 128, :], yg[:])

                yg_full = nc.dram_tensor(f"yg_full_{c}", [DI, TC], BF16, addr_space="Shared")
                nc.gpsimd.collective_compute(
                    "AllGather", OP.bypass, replica_groups=[list(range(NC))],
                    ins=[yg_part[:]], outs=[yg_full.ap()])
                ctx[c]["yg_full"] = yg_full

            def tail(c):
                """gather yg_full, out projection, bias, store."""
                yg_full = ctx[c]["yg_full"]
                ps_o = psB.tile([EL, TC], F32, tag="pso")
                for half in range(2):
                    ygs = xp.tile([128, 8 * TC], BF16, tag="ygs")
                    for j in range(8):
                        kt = half * 8 + j
                        nc.sync.dma_start(ygs[:, j * TC:(j + 1) * TC],
                                          yg_full.ap()[kt * 128:(kt + 1) * 128, :])
                    for j in range(8):
                        kt = half * 8 + j
                        nc.tensor.matmul(ps_o[:], wc[:, kt * EL:(kt + 1) * EL],
                                         ygs[:, j * TC:(j + 1) * TC],
                                         start=(kt == 0), stop=(kt == 15))
                o_sb = wk.tile([EL, TC], F32, tag="osb")
                nc.scalar.activation(o_sb[:], ps_o[:], AF.Identity, bias=bo[:])
                nc.sync.dma_start(out[c, :, :], o_sb[:])
                del ctx[c]

            for c in range(NCH + 3):
                if c < NCH:
                    front(c)
                if c >= 2 and c - 2 < NCH:
                    mid(c - 2)
                if c >= 3:
                    tail(c - 3)

    nc.compile()
    _NC_CACHE["nc"] = nc
    return nc


def _prep_inputs(inputs):
    x = np.ascontiguousarray(np.asarray(inputs["x"], np.float32))
    W_in = np.asarray(inputs["W_in"], np.float32)
    conv_w = np.asarray(inputs["conv_w"], np.float32)
    conv_b = np.asarray(inputs["conv_b"], np.float32)
    W_xp = np.asarray(inputs["W_xp"], np.float32)
    W_dt = np.asarray(inputs["W_dt"], np.float32)
    b_dt = np.asarray(inputs["b_dt"], np.float32)
    A_log = np.asarray(inputs["A_log"], np.float32)
    D = np.asarray(inputs["D"], np.float32)
    W_out_ssm = np.asarray(inputs["W_out_ssm"], np.float32)
    W_out = np.asarray(inputs["W_out"], np.float32)
    b_out = np.asarray(inputs["b_out"], np.float32)

    A = -np.exp(A_log)
    W_c = (W_out.astype(np.float64) @ W_out_ssm.astype(np.float64)).astype(np.float32)
    x_t = np.ascontiguousarray(x.reshape(NT, DM).T.astype(BFnp))  # [DM, NT] bf16
    sel_np = np.zeros((S, S * 128), BFnp)
    for s in range(S):
        sel_np[s, s * 128:(s + 1) * 128] = 1.0

    in_maps = []
    for k in range(NC):
        dsl = slice(k * DIL, (k + 1) * DIL)
        esl = slice(k * EL, (k + 1) * EL)
        in_maps.append({
            "x_t": x_t,
            "w_in_x": np.ascontiguousarray(W_in[dsl, :].T.astype(BFnp)),
            "w_in_z": np.ascontiguousarray(
                W_in[DI + k * DIL: DI + (k + 1) * DIL, :].T.astype(BFnp)),
            "conv_w": np.ascontiguousarray(conv_w[dsl, 0, :]),
            "conv_b": np.ascontiguousarray(conv_b[dsl][:, None]),
            "w_xp": np.ascontiguousarray(W_xp[:, dsl].T.astype(BFnp)),
            "w_dt": np.ascontiguousarray(W_dt[dsl, :].T.astype(BFnp)),
            "b_dt": np.ascontiguousarray(b_dt[dsl][:, None]),
            "a_mat": np.ascontiguousarray(A[dsl, :]),
            "d_vec": np.ascontiguousarray(D[dsl][:, None]),
            "w_c": np.ascontiguousarray(W_c[esl, :].T.astype(BFnp)),
            "b_o": np.ascontiguousarray(b_out[esl][:, None]),
            "sel_in": sel_np,
        })
    return in_maps


def _assemble(results):
    full = np.zeros((DM, NT), np.float32)
    for k in range(NC):
        o = results[k]["out"]  # [NCH, EL, TC]
        for c in range(NCH):
            full[k * EL:(k + 1) * EL, c * TC:(c + 1) * TC] = o[c]
    return np.ascontiguousarray(full.T).reshape(B, L, DM)


def kernel(**inputs):
    nc = build()
    in_maps = _prep_inputs(inputs)
    res = run_bass_kernel_spmd(nc, in_maps, core_ids=list(range(NC)))
    return _assemble(res.results)


def kernel_sim(**inputs):
    """Run through MultiCoreSim instead of HW (for debugging)."""
    from concourse.bass_interp import MultiCoreSim
    nc = build()
    in_maps = _prep_inputs(inputs)
    sim = MultiCoreSim(nc, num_cores=NC)
    for k in range(NC):
        for name, arr in in_maps[k].items():
            sim.cores[k].tensor(name)[:] = arr
    sim.simulate(check_with_hw=False)
    results = [{"out": sim.cores[k].tensor("out").copy()} for k in range(NC)]
    return _assemble(results)



# revision 12
# speedup vs baseline: 1.3909x; 1.3909x over previous
"""Mamba SSM block on 8 TRN2 NeuronCores (Bass/Tile, SPMD).

Sharding: d_inner (2048 -> 256/core) for in_proj, conv, dt/B/C projections and
the selective scan; output projection is token-sharded (512 tokens/core).
Collectives per run:
  - AllReduce of x_dbl projection partials [96, 512] bf16 per 512-token chunk
  - Three AllToAlls of the gated scan output yg (chunks 0-3 / 4-5 / 6-7) that
    reshard yg from d_inner-sharded to token-sharded; each core then computes
    the full (host-folded) W_c = W_out @ W_out_ssm projection for its tokens.

Scan: h[t] = exp(A dt[t]) h[t-1] + (dt[t] x[t]) B[t] via 16 tensor_tensor_scan
ops (one per state index) along tokens, chained across chunks through
per-partition `initial` APs. B/C are broadcast to 128 partitions by a single
stride-0-partition DMA per chunk (no PE/ACT involvement). silu/softplus run
natively on the ACT engine. Matmul operands and scan tensors are bf16
(rel_err ~5e-3 validated vs fp32 reference); conv, dt and the AllReduce stay
fp32.
"""
import numpy as np
import ml_dtypes

import concourse.bass as bass
import concourse.tile as tile
from concourse import bacc, mybir
from concourse.bass_utils import run_bass_kernel_spmd

BFnp = ml_dtypes.bfloat16
F32 = mybir.dt.float32
BF16 = mybir.dt.bfloat16
AF = mybir.ActivationFunctionType
OP = mybir.AluOpType

NC = 8
B, L, DM = 2, 2048, 1024
DI, S, R, KC = 2048, 16, 64, 4
DIL = DI // NC            # 256 d_inner per core
NT = B * L                # 4096 tokens (batch-major)
TC = 512                  # tokens per chunk
NCH = NT // TC            # 8 chunks
NI = DIL // 128           # 2 partition tiles of local d_inner
EO = 8                    # 1024 output rows = 8 tiles of 128

# AllToAll groups: (chunks, tokens-per-block)
A2A_GROUPS = [((0, 1, 2, 3), 256), ((4, 5), 128), ((6, 7), 128)]

_NC_CACHE = {}


def build(use_silu=True):
    """use_silu=False replaces native Silu with Sigmoid+mult (CoreSim lacks Silu)."""
    key = ("nc", use_silu)
    if key in _NC_CACHE:
        return _NC_CACHE[key]
    nc = bacc.Bacc("TRN2", target_bir_lowering=False, debug=False, num_devices=NC)
    _NC_CACHE[key] = None  # placeholder

    # ---- per-core DRAM inputs (host pre-sharded / transposed / casted) ----
    x_t = nc.dram_tensor("x_t", [DM, NT], BF16, kind="ExternalInput")         # replicated
    w_in_x = nc.dram_tensor("w_in_x", [DM, DIL], BF16, kind="ExternalInput")  # W_in[dk,:].T
    w_in_z = nc.dram_tensor("w_in_z", [DM, DIL], BF16, kind="ExternalInput")
    conv_w = nc.dram_tensor("conv_w", [DIL, KC], F32, kind="ExternalInput")
    conv_b = nc.dram_tensor("conv_b", [DIL, 1], F32, kind="ExternalInput")
    w_xp = nc.dram_tensor("w_xp", [DIL, R + 2 * S], BF16, kind="ExternalInput")  # W_xp[:,dk].T
    w_dt = nc.dram_tensor("w_dt", [R, DIL], BF16, kind="ExternalInput")          # W_dt[dk,:].T
    b_dt = nc.dram_tensor("b_dt", [DIL, 1], F32, kind="ExternalInput")
    a_mat = nc.dram_tensor("a_mat", [DIL, S], F32, kind="ExternalInput")        # -exp(A_log[dk])
    d_vec = nc.dram_tensor("d_vec", [DIL, 1], F32, kind="ExternalInput")
    w_c = nc.dram_tensor("w_c", [128, 16 * EO * 128], BF16, kind="ExternalInput")  # W_c blocks, replicated
    b_o = nc.dram_tensor("b_o", [128, EO], F32, kind="ExternalInput")
    out = nc.dram_tensor("out", [8 * 128, TC], F32, kind="ExternalOutput")     # [1024, my 512 tokens]

    with tile.TileContext(nc) as tc:
        with (
            tc.tile_pool(name="wpool", bufs=1) as wp,     # persistent weights
            tc.tile_pool(name="xckp", bufs=2) as xkp,     # streamed x
            tc.tile_pool(name="ygsp", bufs=1) as ygp,     # A2A output gather
            tc.tile_pool(name="work", bufs=1) as wk,      # DVE-only transients
            tc.tile_pool(name="worka", bufs=2) as wka,    # ACT/DMA-written tiles
            tc.tile_pool(name="keep", bufs=6) as kp,      # xs/g (live across chunk)
            tc.tile_pool(name="scan", bufs=1) as sc,      # big bf16 scan tiles
            tc.tile_pool(name="scana", bufs=1) as sca,    # a_t
            tc.tile_pool(name="bcast", bufs=1) as bcp,    # B/C broadcast tiles
            tc.tile_pool(name="state", bufs=1) as st,     # persistent hprev/xtail
            tc.tile_pool(name="psA", bufs=2, space="PSUM") as psA,
            tc.tile_pool(name="psB1", bufs=1, space="PSUM") as psB1,
            tc.tile_pool(name="psB2", bufs=2, space="PSUM") as psB2,
            tc.tile_pool(name="psO", bufs=1, space="PSUM") as psO,
            tc.tile_pool(name="dram", bufs=1, space="DRAM") as dr,
        ):
            # ---------- load weights ----------
            winx = wp.tile([128, 8 * NI * 128], BF16, tag="winx")
            winz = wp.tile([128, 8 * NI * 128], BF16, tag="winz")
            nc.sync.dma_start(
                winx[:].rearrange("p (k i n) -> p k i n", k=8, i=NI),
                w_in_x.ap().rearrange("(k p) (i n) -> p k i n", p=128, i=NI))
            nc.sync.dma_start(
                winz[:].rearrange("p (k i n) -> p k i n", k=8, i=NI),
                w_in_z.ap().rearrange("(k p) (i n) -> p k i n", p=128, i=NI))
            wxp = wp.tile([128, NI * (R + 2 * S)], BF16, tag="wxp")
            nc.sync.dma_start(
                wxp[:].rearrange("p (i n) -> p i n", i=NI),
                w_xp.ap().rearrange("(i p) n -> p i n", p=128))
            wdt = wp.tile([R, NI * 128], BF16, tag="wdt")
            nc.sync.dma_start(wdt[:], w_dt[:, :])
            wc = wp.tile([128, 16 * EO * 128], BF16, tag="wc")
            nc.sync.dma_start(wc[:], w_c[:, :])
            cw = wp.tile([128, NI * KC], F32, tag="cw")
            cb = wp.tile([128, NI], F32, tag="cb")
            bdt = wp.tile([128, NI], F32, tag="bdt")
            dv = wp.tile([128, NI], F32, tag="dv")
            am = wp.tile([128, NI * S], F32, tag="am")
            for i in range(NI):
                sl = slice(i * 128, (i + 1) * 128)
                nc.sync.dma_start(cw[:, i * KC:(i + 1) * KC], conv_w[sl, :])
                nc.sync.dma_start(cb[:, i:i + 1], conv_b[sl, :])
                nc.sync.dma_start(bdt[:, i:i + 1], b_dt[sl, :])
                nc.sync.dma_start(dv[:, i:i + 1], d_vec[sl, :])
                nc.sync.dma_start(am[:, i * S:(i + 1) * S], a_mat[sl, :])
            bo = wp.tile([128, EO], F32, tag="bo")
            nc.sync.dma_start(bo[:], b_o[:, :])

            hprev = st.tile([128, NI * S], BF16, tag="hprev")
            xtail = st.tile([128, NI * 3], F32, tag="xtail")

            # A2A input/output DRAM tiles (local, not Shared)
            a2a_in = [dr.tile([2048, tb], BF16, tag=f"a2ain{g}", name=f"a2ain{g}")
                      for g, (_, tb) in enumerate(A2A_GROUPS)]
            a2a_out = [dr.tile([2048, tb], BF16, tag=f"a2aout{g}", name=f"a2aout{g}")
                       for g, (_, tb) in enumerate(A2A_GROUPS)]

            ctx = {}

            def front(c):
                """x stream, in_proj, conv, silu, z-gate, x_dbl partial, AR issue."""
                t0 = c * TC
                reset = (c % (NCH // B) == 0)

                xck = xkp.tile([128, 8 * TC], BF16, tag="xck")
                nc.sync.dma_start(
                    xck[:].rearrange("p (k t) -> p k t", k=8),
                    x_t.ap()[:, t0:t0 + TC].rearrange("(k p) t -> p k t", p=128))

                xs_i, g_i = [], []
                for i in range(NI):
                    ps_x = psA.tile([128, TC], F32, tag="psx")
                    ps_z = psA.tile([128, TC], F32, tag="psz")
                    for kt in range(8):
                        wsl = slice((kt * NI + i) * 128, (kt * NI + i + 1) * 128)
                        nc.tensor.matmul(ps_x[:], winx[:, wsl], xck[:, kt * TC:(kt + 1) * TC],
                                         start=(kt == 0), stop=(kt == 7))
                    for kt in range(8):
                        wsl = slice((kt * NI + i) * 128, (kt * NI + i + 1) * 128)
                        nc.tensor.matmul(ps_z[:], winz[:, wsl], xck[:, kt * TC:(kt + 1) * TC],
                                         start=(kt == 0), stop=(kt == 7))

                    # causal depthwise conv (taps read PSUM)
                    head = wk.tile([128, 7], F32, tag="head")
                    if reset:
                        nc.gpsimd.memset(head[:, 0:3], 0.0)
                    else:
                        nc.vector.tensor_copy(head[:, 0:3], xtail[:, i * 3:i * 3 + 3])
                    nc.vector.tensor_copy(head[:, 3:7], ps_x[:, 0:4])
                    nc.vector.tensor_copy(xtail[:, i * 3:i * 3 + 3], ps_x[:, TC - 3:TC])

                    acc0 = wk.tile([128, TC], F32, tag="accA")
                    nc.vector.tensor_scalar_mul(acc0[:, 3:], ps_x[:, 0:TC - 3], cw[:, i * KC:i * KC + 1])
                    acc1 = wk.tile([128, TC], F32, tag="accB")
                    nc.vector.scalar_tensor_tensor(
                        out=acc1[:, 3:], in0=ps_x[:, 1:TC - 2], scalar=cw[:, i * KC + 1:i * KC + 2],
                        in1=acc0[:, 3:], op0=OP.mult, op1=OP.add)
                    acc2 = wk.tile([128, TC], F32, tag="accA")
                    nc.vector.scalar_tensor_tensor(
                        out=acc2[:, 3:], in0=ps_x[:, 2:TC - 1], scalar=cw[:, i * KC + 2:i * KC + 3],
                        in1=acc1[:, 3:], op0=OP.mult, op1=OP.add)
                    u = wk.tile([128, TC], F32, tag="accB")
                    nc.vector.scalar_tensor_tensor(
                        out=u[:, 3:], in0=ps_x[:, 3:TC], scalar=cw[:, i * KC + 3:i * KC + 4],
                        in1=acc2[:, 3:], op0=OP.mult, op1=OP.add)
                    nc.vector.tensor_scalar_mul(acc0[:, 0:3], head[:, 0:3], cw[:, i * KC:i * KC + 1])
                    nc.vector.scalar_tensor_tensor(
                        out=acc1[:, 0:3], in0=head[:, 1:4], scalar=cw[:, i * KC + 1:i * KC + 2],
                        in1=acc0[:, 0:3], op0=OP.mult, op1=OP.add)
                    nc.vector.scalar_tensor_tensor(
                        out=acc2[:, 0:3], in0=head[:, 2:5], scalar=cw[:, i * KC + 2:i * KC + 3],
                        in1=acc1[:, 0:3], op0=OP.mult, op1=OP.add)
                    nc.vector.scalar_tensor_tensor(
                        out=u[:, 0:3], in0=head[:, 3:6], scalar=cw[:, i * KC + 3:i * KC + 4],
                        in1=acc2[:, 0:3], op0=OP.mult, op1=OP.add)
                    xs = kp.tile([128, TC], BF16, tag="xs")
                    g = kp.tile([128, TC], BF16, tag="g")
                    if use_silu:
                        nc.scalar.activation(xs[:], u[:], AF.Silu, bias=cb[:, i:i + 1])
                        # z gate: g = silu(z) straight from PSUM
                        nc.scalar.activation(g[:], ps_z[:], AF.Silu)
                    else:
                        sgu = wka.tile([128, TC], F32, tag="sgu")
                        nc.scalar.activation(sgu[:], u[:], AF.Sigmoid, bias=cb[:, i:i + 1])
                        nc.vector.scalar_tensor_tensor(
                            out=xs[:], in0=u[:], scalar=cb[:, i:i + 1], in1=sgu[:],
                            op0=OP.add, op1=OP.mult)
                        sgz = wka.tile([128, TC], F32, tag="sgz")
                        nc.scalar.activation(sgz[:], ps_z[:], AF.Sigmoid)
                        nc.vector.scalar_tensor_tensor(
                            out=g[:], in0=sgz[:], scalar=1.0, in1=ps_z[:],
                            op0=OP.mult, op1=OP.mult)
                    xs_i.append(xs)
                    g_i.append(g)

                # x_dbl partial + AllReduce
                ps_xd = psB1.tile([R + 2 * S, TC], F32, tag="psxd")
                for i in range(NI):
                    nc.tensor.matmul(ps_xd[:], wxp[:, i * (R + 2 * S):(i + 1) * (R + 2 * S)],
                                     xs_i[i][:], start=(i == 0), stop=(i == NI - 1))
                xd_sb = wka.tile([R + 2 * S, TC], BF16, tag="xdsb")
                nc.scalar.copy(xd_sb[:], ps_xd[:])
                xd_part = dr.tile([R + 2 * S, TC], BF16, tag=f"xdp{c % 4}")
                nc.sync.dma_start(xd_part[:], xd_sb[:])
                xd_red = nc.dram_tensor(f"xd_red_{c}", [R + 2 * S, TC], BF16, addr_space="Shared")
                nc.gpsimd.collective_compute(
                    "AllReduce", OP.add, replica_groups=[list(range(NC))],
                    ins=[xd_part[:]], outs=[xd_red.ap()])
                ctx[c] = dict(xs_i=xs_i, g_i=g_i, xd_red=xd_red)

            def mid(c):
                """dtr/B/C loads, dt, a-exps, bb, scan, hc, tree, gate, yg store."""
                reset = (c % (NCH // B) == 0)
                xs_i = ctx[c]["xs_i"]
                g_i = ctx[c]["g_i"]
                xd_red = ctx[c]["xd_red"]

                dtr = wka.tile([R, TC], BF16, tag="dtr")
                nc.sync.dma_start(dtr[:], xd_red.ap()[0:R, :])
                # B/C rows broadcast to all 128 partitions: one stride-0 DMA each
                b_bc = bcp.tile([128, S * TC], BF16, tag="bbc")
                c_bc = bcp.tile([128, S * TC], BF16, tag="cbc")
                nc.scalar.dma_start(
                    b_bc[:].rearrange("p (s t) -> p s t", s=S),
                    xd_red.ap()[R:R + S, :].unsqueeze(0).broadcast_to([128, S, TC]))
                nc.scalar.dma_start(
                    c_bc[:].rearrange("p (s t) -> p s t", s=S),
                    xd_red.ap()[R + S:R + 2 * S, :].unsqueeze(0).broadcast_to([128, S, TC]))

                # find the A2A group/block layout for this chunk
                for g, (chunks, tb) in enumerate(A2A_GROUPS):
                    if c in chunks:
                        grp, tb_g, coff = g, tb, chunks.index(c) * (TC // tb)
                        break

                # dt for both i-tiles first; Exp+Ln softplus keeps the whole mid
                # phase on the natural_log_exp activation table (a_t uses Exp too)
                dt_i = []
                for i in range(NI):
                    ps_dt = psB2.tile([128, TC], F32, tag="psdt")
                    nc.tensor.matmul(ps_dt[:], wdt[:, i * 128:(i + 1) * 128], dtr[:],
                                     start=True, stop=True)
                    edt = wka.tile([128, TC], F32, tag="edt")
                    nc.scalar.activation(edt[:], ps_dt[:], AF.Exp, bias=bdt[:, i:i + 1])
                    dt = wka.tile([128, TC], F32, tag="dtt")
                    nc.scalar.activation(dt[:], edt[:], AF.Ln, bias=1.0)
                    dt_i.append(dt)

                for i in range(NI):
                    dt = dt_i[i]
                    dtx = wk.tile([128, TC], BF16, tag="dtx")
                    nc.vector.tensor_tensor(out=dtx[:], in0=dt[:], in1=xs_i[i][:], op=OP.mult)

                    a_t = sca.tile([128, S * TC], BF16, tag="a_t")
                    for s in range(S):
                        nc.scalar.activation(
                            a_t[:, s * TC:(s + 1) * TC], dt[:], AF.Exp,
                            scale=am[:, i * S + s:i * S + s + 1])

                    bb_t = sc.tile([128, S * TC], BF16, tag="bb_t")
                    nc.vector.tensor_tensor(
                        out=bb_t[:].rearrange("p (s t) -> p s t", s=S),
                        in0=dtx[:].unsqueeze(1).broadcast_to([128, S, TC]),
                        in1=b_bc[:].rearrange("p (s t) -> p s t", s=S), op=OP.mult)

                    h_t = sc.tile([128, S * TC], BF16, tag="h_t")
                    if reset:
                        nc.gpsimd.memset(hprev[:, i * S:(i + 1) * S], 0.0)
                    for s in range(S):
                        nc.vector.tensor_tensor_scan(
                            h_t[:, s * TC:(s + 1) * TC],
                            a_t[:, s * TC:(s + 1) * TC],
                            bb_t[:, s * TC:(s + 1) * TC],
                            hprev[:, i * S + s:i * S + s + 1],
                            op0=OP.mult, op1=OP.add)
                    nc.vector.tensor_copy(
                        hprev[:, i * S:(i + 1) * S],
                        h_t[:].rearrange("p (s t) -> p s t", s=S)[:, :, TC - 1])

                    hc_t = sc.tile([128, S * TC], BF16, tag="hc_t")
                    nc.vector.tensor_tensor(out=hc_t[:], in0=h_t[:], in1=c_bc[:], op=OP.mult)
                    r1 = sc.tile([128, S * TC // 2], BF16, tag="bb_t")
                    nc.gpsimd.tensor_tensor(out=r1[:], in0=hc_t[:, :S * TC // 2],
                                            in1=hc_t[:, S * TC // 2:], op=OP.add)
                    r2 = sc.tile([128, S * TC // 4], BF16, tag="h_t")
                    nc.gpsimd.tensor_tensor(out=r2[:], in0=r1[:, :S * TC // 4],
                                            in1=r1[:, S * TC // 4:], op=OP.add)
                    r3 = sc.tile([128, S * TC // 8], BF16, tag="bb_t")
                    nc.vector.tensor_tensor(out=r3[:], in0=r2[:, :S * TC // 8],
                                            in1=r2[:, S * TC // 8:], op=OP.add)
                    y = wk.tile([128, TC], F32, tag="y")
                    nc.vector.tensor_tensor(out=y[:], in0=r3[:, :TC], in1=r3[:, TC:], op=OP.add)

                    yD = wk.tile([128, TC], F32, tag="yD")
                    nc.vector.scalar_tensor_tensor(
                        out=yD[:], in0=xs_i[i][:], scalar=dv[:, i:i + 1], in1=y[:],
                        op0=OP.mult, op1=OP.add)
                    yg = wk.tile([128, TC], BF16, tag="yg")
                    nc.vector.tensor_tensor(out=yg[:], in0=yD[:], in1=g_i[i][:], op=OP.mult)
                    # scatter yg into the A2A input: row-block = token-block,
                    # rows within block = my local d channels
                    for q in range(TC // tb_g):
                        blk = coff + q
                        nc.sync.dma_start(
                            a2a_in[grp][blk * 256 + i * 128: blk * 256 + (i + 1) * 128, :],
                            yg[:, q * tb_g:(q + 1) * tb_g])
                del ctx[c]["xs_i"], ctx[c]["g_i"]

            def a2a(g):
                nc.gpsimd.collective_compute(
                    "AllToAll", OP.bypass, replica_groups=[list(range(NC))],
                    ins=[a2a_in[g][:]], outs=[a2a_out[g][:]])

            def tail(g):
                """gather a2a_out[g], out projection, bias, store my token cols."""
                chunks, tb = A2A_GROUPS[g]
                col0 = sum(t for _, t in A2A_GROUPS[:g])
                ygs = ygp.tile([128, 16 * tb], BF16, tag="ygs")
                nc.sync.dma_start(
                    ygs[:].rearrange("p (k t) -> p k t", k=16),
                    a2a_out[g][:, :].rearrange("(k p) t -> p k t", p=128))
                for e in range(EO):
                    ps_o = psO.tile([128, tb], F32, tag="pso")
                    for kt in range(16):
                        nc.tensor.matmul(ps_o[:], wc[:, (kt * EO + e) * 128:(kt * EO + e + 1) * 128],
                                         ygs[:, kt * tb:(kt + 1) * tb],
                                         start=(kt == 0), stop=(kt == 15))
                    o_sb = wka.tile([128, tb], F32, tag="osb")
                    nc.scalar.activation(o_sb[:], ps_o[:], AF.Identity, bias=bo[:, e:e + 1])
                    nc.sync.dma_start(out[e * 128:(e + 1) * 128, col0:col0 + tb], o_sb[:])

            for c in range(NCH + 3):
                if c < NCH:
                    front(c)
                if 2 <= c < NCH + 2:
                    mid(c - 2)
                    if c - 2 == 3:
                        a2a(0)
                    elif c - 2 == 5:
                        a2a(1)
                    elif c - 2 == 7:
                        a2a(2)
                if c == 7:
                    tail(0)
                elif c == 9:
                    tail(1)
                elif c == 10:
                    tail(2)

    nc.compile()
    _NC_CACHE[key] = nc
    return nc


def _prep_inputs(inputs):
    x = np.ascontiguousarray(np.asarray(inputs["x"], np.float32))
    W_in = np.asarray(inputs["W_in"], np.float32)
    conv_w = np.asarray(inputs["conv_w"], np.float32)
    conv_b = np.asarray(inputs["conv_b"], np.float32)
    W_xp = np.asarray(inputs["W_xp"], np.float32)
    W_dt = np.asarray(inputs["W_dt"], np.float32)
    b_dt = np.asarray(inputs["b_dt"], np.float32)
    A_log = np.asarray(inputs["A_log"], np.float32)
    D = np.asarray(inputs["D"], np.float32)
    W_out_ssm = np.asarray(inputs["W_out_ssm"], np.float32)
    W_out = np.asarray(inputs["W_out"], np.float32)
    b_out = np.asarray(inputs["b_out"], np.float32)

    A = -np.exp(A_log)
    W_c = (W_out.astype(np.float64) @ W_out_ssm.astype(np.float64)).astype(np.float32)
    x_t = np.ascontiguousarray(x.reshape(NT, DM).T.astype(BFnp))  # [DM, NT] bf16

    # W_c as stationary blocks: [128, (kt*EO+e)*128] = W_c[e-rows, kt-cols].T
    wc_blk = np.zeros((128, 16 * EO * 128), BFnp)
    for kt in range(16):
        for e in range(EO):
            blk = W_c[e * 128:(e + 1) * 128, kt * 128:(kt + 1) * 128].T
            wc_blk[:, (kt * EO + e) * 128:(kt * EO + e + 1) * 128] = blk.astype(BFnp)
    bo_blk = np.ascontiguousarray(b_out.reshape(EO, 128).T)  # [128, EO]

    in_maps = []
    for k in range(NC):
        dsl = slice(k * DIL, (k + 1) * DIL)
        in_maps.append({
            "x_t": x_t,
            "w_in_x": np.ascontiguousarray(W_in[dsl, :].T.astype(BFnp)),
            "w_in_z": np.ascontiguousarray(
                W_in[DI + k * DIL: DI + (k + 1) * DIL, :].T.astype(BFnp)),
            "conv_w": np.ascontiguousarray(conv_w[dsl, 0, :]),
            "conv_b": np.ascontiguousarray(conv_b[dsl][:, None]),
            "w_xp": np.ascontiguousarray(W_xp[:, dsl].T.astype(BFnp)),
            "w_dt": np.ascontiguousarray(W_dt[dsl, :].T.astype(BFnp)),
            "b_dt": np.ascontiguousarray(b_dt[dsl][:, None]),
            "a_mat": np.ascontiguousarray(A[dsl, :]),
            "d_vec": np.ascontiguousarray(D[dsl][:, None]),
            "w_c": wc_blk,
            "b_o": bo_blk,
        })
    return in_maps


def _assemble(results):
    full = np.zeros((DM, NT), np.float32)
    for k in range(NC):
        o = results[k]["out"]  # [1024, 512]: A(256) | B(128) | C(128) token cols
        full[:, 256 * k: 256 * k + 256] = o[:, 0:256]
        full[:, 2048 + 128 * k: 2048 + 128 * k + 128] = o[:, 256:384]
        full[:, 3072 + 128 * k: 3072 + 128 * k + 128] = o[:, 384:512]
    return np.ascontiguousarray(full.T).reshape(B, L, DM)


def kernel(**inputs):
    nc = build()
    in_maps = _prep_inputs(inputs)
    res = run_bass_kernel_spmd(nc, in_maps, core_ids=list(range(NC)))
    return _assemble(res.results)


def kernel_sim(**inputs):
    """Run through MultiCoreSim instead of HW (for debugging)."""
    from concourse.bass_interp import MultiCoreSim
    nc = build(use_silu=False)
    in_maps = _prep_inputs(inputs)
    sim = MultiCoreSim(nc, num_cores=NC)
    for k in range(NC):
        for name, arr in in_maps[k].items():
            sim.cores[k].tensor(name)[:] = arr
    sim.simulate(check_with_hw=False)
    results = [{"out": sim.cores[k].tensor("out").copy()} for k in range(NC)]
    return _assemble(results)
